# revision 12
# baseline (speedup 1.0000x reference)
"""Trainium2 Bass kernel for a CrossAttentionBlock.

Per-core computation (data-parallel over batch, B=8 -> 8 NeuronCores):
  qc   = conv2d_3x3_same(q, conv_w)                  [64, 48, 48]
  qp   = (rmsnorm(qc) @ wq.T + bq) / 4               per-pixel RMS over C
  kp   = rmsnorm(k) @ wk.T + bk
  per head h (4 heads, d=16):  S_h[i,j] = qp_h qp... = qp_h^T kp_h
  out  = 0.25 * sum_h softmax_j(S_h) @ v^T           [2304, 64]

v2 architecture ("transposed-scores"):
  - scores are computed as [i (partitions), j (free)] so the softmax
    denominator Z_h[i] is a per-partition scalar: it falls out of the
    activation instruction's accum_out during the exp pass itself
  - the per-head exp matrices E_h are scaled by 1/Z_h and summed over
    heads on the vector engine (scalar_tensor_tensor chain) into ONE
    matrix P[i,j] -- so the PV matmul runs once, not once per head
  - P is transposed back to [j, i] by the DMA XBAR transpose engine
    (idle resource; 14ns per 16x128 tile), then a single accumulated
    PV matmul produces out^T = (0.25*v)^T P^T
  - the last 256 columns of each exp row run on the vector engine via
    the Schraudolph bit-trick (int32 <-> f32 exponent hack) to offload
    the scalar engine, which is otherwise the bottleneck
  - rsqrt for RMS via exp(-0.5 ln x) so preamble+main share one ACT
    table set
"""

import sys
import types

for _p in ("/opt/trn_rl_repo", "/root/.axon_site"):
    if _p not in sys.path:
        sys.path.insert(0, _p)

import numpy as np


def _ensure_ntff_hook():
    """Register the axon NTFF profile hook if the image's antenv lacks it."""
    try:
        import antenv.axon_hooks  # noqa: F401

        return
    except ImportError:
        pass
    try:
        from trn_agent_boot.trn_boot import _ntff_profile_via_ctypes

        hook = _ntff_profile_via_ctypes("/opt/axon/libaxon_pjrt.so")
        mod = types.ModuleType("antenv.axon_hooks")
        mod.get_axon_ntff_profile_hook = lambda: hook
        mod.set_axon_ntff_profile_hook = lambda h: None
        sys.modules["antenv.axon_hooks"] = mod
    except Exception:
        pass


_ensure_ntff_hook()

import concourse.bacc as bacc
import concourse.tile as tile
from concourse import mybir
from concourse.bass_utils import run_bass_kernel_spmd
from concourse.masks import make_identity

F32 = mybir.dt.float32
F32R = mybir.dt.float32r
BF16 = mybir.dt.bfloat16
I32 = mybir.dt.int32
AF = mybir.ActivationFunctionType
OP = mybir.AluOpType
AX = mybir.AxisListType

N_CORES = 8
C = 64
H = W = 48
S = H * W  # 2304
NH = 4
HD = 16
SCALE = 1.0 / np.sqrt(HD)  # 0.25
EPS = 1.1920929e-07
WP = W + 2  # padded width 50
NJT = S // 128  # 18 j-tiles (128 wide)
NIB = S // 128  # 18 i-blocks (128 wide)
# preamble projection chunks: 4 x 512 + 1 x 256
ICHUNKS = [(0, 512), (512, 512), (1024, 512), (1536, 512), (2048, 256)]
# main-loop i chunks for the PV matmul
PVCH = [(0, 512), (512, 512), (1024, 512), (1536, 512), (2048, 256)]
# Schraudolph exp: bitcast_f32(int32(A*x + B)); tail columns on DVE
A32 = float(2**23) / float(np.log(2.0))  # 12102203.16
B32 = float(127 * 2**23 - 458752)  # mantissa-bias constant, c=0.0547
JTAIL = 256  # j columns per row handled on the vector engine
JACT = S - JTAIL  # 2048, as two [128,1024] activation instructions


def build(stage=None):
    import os

    stage = stage or os.environ.get("K_STAGE", "full")
    nc = bacc.Bacc(
        "TRN2", target_bir_lowering=False, debug=False, num_devices=N_CORES
    )

    q_d = nc.dram_tensor("q", [C, S], F32, kind="ExternalInput").ap()
    k_d = nc.dram_tensor("k", [C, S], F32, kind="ExternalInput").ap()
    v_d = nc.dram_tensor("v", [C, S], F32, kind="ExternalInput").ap()
    cw_d = nc.dram_tensor("conv_w", [C, C * 9], F32, kind="ExternalInput").ap()
    wq_d = nc.dram_tensor("wq", [C, C], F32, kind="ExternalInput").ap()
    wk_d = nc.dram_tensor("wk", [C, C], F32, kind="ExternalInput").ap()
    bq_d = nc.dram_tensor("bq", [C, 1], F32, kind="ExternalInput").ap()
    bk_d = nc.dram_tensor("bk", [C, 1], F32, kind="ExternalInput").ap()
    nq_d = nc.dram_tensor("nq_w", [C, 1], F32, kind="ExternalInput").ap()
    nk_d = nc.dram_tensor("nk_w", [C, 1], F32, kind="ExternalInput").ap()
    # out^T: [d, i]; host transposes back
    out_d = nc.dram_tensor("out", [C, S], F32, kind="ExternalOutput").ap()

    with tile.TileContext(nc) as tc:
        with tc.tile_pool(name="const", bufs=1) as const:
            # ---- persistent (main-loop) tiles ----
            kp_pack = const.tile([128, S], BF16)  # head h at parts 32h..+15
            qp_all = const.tile([128, NH * S], BF16)  # per-head zero-pad slabs
            vT_sb = const.tile([128, NJT * C], BF16)  # 0.25 * v^T per j-tile
            accT = const.tile([C, S], F32)  # out^T accumulator
            zero_dep = const.tile([128, 1], F32)  # Exp-after-Ln dep bias

            with tc.tile_pool(name="pre", bufs=1) as pre:
                # ---------------- input DMAs (spread over queues) --------
                q_in = pre.tile([C, S], F32)
                k_in = pre.tile([C, S], F32)
                v_in = pre.tile([C, S], F32)
                cw_sb = pre.tile([C, C * 9], F32)
                wq_sb = pre.tile([C, C], F32)
                wk_sb = pre.tile([C, C], F32)
                bq_col = pre.tile([C, 1], F32)
                bk_col = pre.tile([C, 1], F32)
                nq_col = pre.tile([C, 1], F32)
                nk_col = pre.tile([C, 1], F32)
                nc.sync.dma_start(out=q_in, in_=q_d)
                nc.scalar.dma_start(out=k_in, in_=k_d)
                nc.gpsimd.dma_start(out=v_in, in_=v_d)
                nc.gpsimd.dma_start(out=cw_sb, in_=cw_d)
                nc.sync.dma_start(out=wq_sb, in_=wq_d)
                nc.sync.dma_start(out=wk_sb, in_=wk_d)
                nc.sync.dma_start(out=bq_col, in_=bq_d)
                nc.sync.dma_start(out=bk_col, in_=bk_d)
                nc.sync.dma_start(out=nq_col, in_=nq_d)
                nc.sync.dma_start(out=nk_col, in_=nk_d)

                # ---------------- constants ----------------
                ident = pre.tile([128, 128], F32)
                make_identity(nc, ident)
                ones_sb = pre.tile([C, 128], BF16)
                nc.gpsimd.memset(ones_sb, 1.0)
                eps_col = pre.tile([128, 1], F32)
                nc.gpsimd.memset(eps_col, EPS)

                # zero-padded conv input (+2 slack for shifted row-slices)
                qpad = pre.tile([C, WP * (H + 2) + 2], BF16)
                qc_sb = pre.tile([C, S], BF16)  # conv output
                qsq_sb = pre.tile([C, S], BF16)  # conv output squared
                ksq_sb = pre.tile([C, S], BF16)
                k_r = pre.tile([C, S], BF16)  # bf16 copy of k
                qp_pack = pre.tile([128, S], BF16)  # spread head staging
                proj_q = pre.tile([128, S], F32)
                proj_k = pre.tile([128, S], F32)
                rln_q = pre.tile([128, S], F32)
                rln_k = pre.tile([128, S], F32)
                r_q = pre.tile([128, S], F32)
                r_k = pre.tile([128, S], F32)
                wq_sp = pre.tile([C, 128], BF16)  # spread projection weights
                wk_sp = pre.tile([C, 128], BF16)
                bq_sp = pre.tile([128, 1], F32)
                bk_sp = pre.tile([128, 1], F32)

                nc.vector.memset(qpad, 0.0)
                nc.vector.memset(qp_all, 0.0)
                nc.gpsimd.memset(wq_sp, 0.0)
                nc.gpsimd.memset(wk_sp, 0.0)
                nc.gpsimd.memset(bq_sp, 0.0)
                nc.gpsimd.memset(bk_sp, 0.0)
                warm_sb = pre.tile([128, 512], BF16)
                nc.vector.memset(warm_sb, 0.125)

                with tc.tile_pool(name="pre_ps", bufs=2, space="PSUM") as pre_ps:
                    # ---- PE warm-up: dense matmuls so the clock gate (HAM)
                    # reaches 2.4 GHz before the real work arrives
                    warm_ps = pre_ps.tile([128, 512], F32, tag="cps")
                    for _ in range(16):
                        nc.tensor.matmul(
                            warm_ps,
                            lhsT=warm_sb[:, 0:128],
                            rhs=warm_sb,
                            start=True,
                            stop=True,
                        )

                    # ---- copy q into padded plane (rows shifted by 1,1) ----
                    dst = qpad[:, WP + 1 : WP + 1 + H * WP].rearrange(
                        "p (r w) -> p r w", w=WP
                    )[:, :, 0:W]
                    nc.vector.tensor_copy(
                        out=dst, in_=q_in.rearrange("p (r w) -> p r w", w=W)
                    )

                    # ---- transpose conv weights: per tap t, [o,i] -> [i,o] --
                    cwT_sb = pre.tile([C, 9 * C], BF16)
                    for t in range(9):
                        tp = pre_ps.tile([128, 65], F32, tag="tps")
                        nc.tensor.transpose(
                            tp[0:C, 0:C],
                            cw_sb.rearrange("p (i t) -> p t i", t=9)[:, t, :],
                            ident[0:C, 0:C],
                        )
                        nc.vector.tensor_copy(
                            out=cwT_sb[:, t * C : (t + 1) * C], in_=tp[0:C, 0:C]
                        )

                    # ---- transpose + fold norm weights into projections ----
                    wqT_sb = pre.tile([C, C], F32)
                    wkT_sb = pre.tile([C, C], F32)
                    for w_sb, wT_sb, n_col in (
                        (wq_sb, wqT_sb, nq_col),
                        (wk_sb, wkT_sb, nk_col),
                    ):
                        tp = pre_ps.tile([128, 65], F32, tag="tps")
                        nc.tensor.transpose(tp[0:C, 0:C], w_sb, ident[0:C, 0:C])
                        nc.vector.tensor_scalar(
                            out=wT_sb,
                            in0=tp[0:C, 0:C],
                            scalar1=n_col,
                            scalar2=None,
                            op0=OP.mult,
                        )
                    # spread head h columns to 32h..32h+15
                    for h in range(NH):
                        nc.vector.tensor_copy(
                            out=wq_sp[:, 32 * h : 32 * h + HD],
                            in_=wqT_sb[:, HD * h : HD * (h + 1)],
                        )
                        nc.vector.tensor_copy(
                            out=wk_sp[:, 32 * h : 32 * h + HD],
                            in_=wkT_sb[:, HD * h : HD * (h + 1)],
                        )
                        # engine partition access must be 32-aligned; DMA
                        nc.gpsimd.dma_start(
                            out=bq_sp[32 * h : 32 * h + HD, :],
                            in_=bq_d[HD * h : HD * (h + 1), :],
                        )
                        nc.gpsimd.dma_start(
                            out=bk_sp[32 * h : 32 * h + HD, :],
                            in_=bk_d[HD * h : HD * (h + 1), :],
                        )
                    # fold the 1/sqrt(head_dim) factor into the q bias
                    nc.vector.tensor_scalar(
                        out=bq_sp, in0=bq_sp, scalar1=SCALE, scalar2=None,
                        op0=OP.mult,
                    )

                    # ---- conv as 9 accumulated shifted matmuls ----
                    row_chunks = [(0, 10), (10, 10), (20, 10), (30, 10), (40, 8)]
                    for r0, nr in row_chunks:
                        cp = pre_ps.tile([C, 512], F32, tag="cps")
                        n_out = nr * W
                        for t in range(9):
                            ky, kx = divmod(t, 3)
                            src = qpad[
                                :,
                                (r0 + ky) * WP + kx : (r0 + ky) * WP
                                + kx
                                + nr * WP,
                            ].rearrange("p (r w) -> p r w", w=WP)[:, :, 0:W]
                            nc.tensor.matmul(
                                cp[:, 0:n_out],
                                lhsT=cwT_sb[:, t * C : (t + 1) * C],
                                rhs=src,
                                start=(t == 0),
                                stop=(t == 8),
                            )
                        sl = slice(r0 * W, r0 * W + n_out)
                        nc.scalar.copy(out=qc_sb[:, sl], in_=cp[:, 0:n_out])
                        nc.vector.tensor_mul(
                            qsq_sb[:, sl], qc_sb[:, sl], qc_sb[:, sl]
                        )

                    # ---- k squared + bf16 k for the projection ----
                    nc.vector.tensor_mul(ksq_sb, k_in, k_in)
                    nc.scalar.copy(out=k_r, in_=k_in)

                    # ---- projections + RMS factors ----
                    # phase 1: matmuls + Ln (all Ln before any Exp)
                    for src_sb, sq_sb, w_sp, proj_f, rln_f in (
                        (k_r, ksq_sb, wk_sp, proj_k, rln_k),
                        (qc_sb, qsq_sb, wq_sp, proj_q, rln_q),
                    ):
                        for c0, cw_ in ICHUNKS:
                            sl = slice(c0, c0 + cw_)
                            pp = pre_ps.tile([128, 512], F32, tag="pps")
                            sp = pre_ps.tile([128, 512], F32, tag="sps")
                            nc.tensor.matmul(
                                pp[:, 0:cw_],
                                lhsT=w_sp,
                                rhs=src_sb[:, sl],
                                start=True,
                                stop=True,
                            )
                            nc.tensor.matmul(
                                sp[:, 0:cw_],
                                lhsT=ones_sb,
                                rhs=sq_sb[:, sl],
                                start=True,
                                stop=True,
                            )
                            nc.scalar.activation(
                                out=rln_f[:, sl],
                                in_=sp[:, 0:cw_],
                                func=AF.Ln,
                                scale=1.0 / C,
                                bias=eps_col,
                            )
                            nc.vector.tensor_copy(
                                out=proj_f[:, sl], in_=pp[:, 0:cw_]
                            )
                    # exp ops bias on this dep tile so every Exp (preamble AND
                    # main loop) is scheduled after every Ln: one table set
                    nc.vector.tensor_mul(
                        zero_dep, rln_q[:, S - 1 : S], rln_k[:, S - 1 : S]
                    )
                    nc.vector.tensor_scalar(
                        out=zero_dep, in0=zero_dep, scalar1=0.0, scalar2=None,
                        op0=OP.mult,
                    )
                    # phases 2+3 per chunk: r = exp(-0.5 ln(mean+eps)); scale,
                    # bias, pack; k first so the main loop can start early
                    for flow, (proj_f, rln_f, r_f, b_sp, dst_pack, post_mul) in (
                        ("k", (proj_k, rln_k, r_k, bk_sp, kp_pack, None)),
                        ("q", (proj_q, rln_q, r_q, bq_sp, qp_pack, SCALE)),
                    ):
                        for c0, cw_ in ICHUNKS:
                            sl = slice(c0, c0 + cw_)
                            nc.scalar.activation(
                                out=r_f[:, sl],
                                in_=rln_f[:, sl],
                                func=AF.Exp,
                                scale=-0.5,
                                bias=zero_dep,
                            )
                            nc.vector.tensor_mul(
                                dst_pack[:, sl], proj_f[:, sl], r_f[:, sl]
                            )
                            if post_mul is not None:
                                nc.vector.tensor_scalar(
                                    out=dst_pack[:, sl],
                                    in0=dst_pack[:, sl],
                                    scalar1=post_mul,
                                    scalar2=b_sp,
                                    op0=OP.mult,
                                    op1=OP.add,
                                )
                            else:
                                nc.vector.tensor_scalar(
                                    out=dst_pack[:, sl],
                                    in0=dst_pack[:, sl],
                                    scalar1=b_sp,
                                    scalar2=None,
                                    op0=OP.add,
                                )
                            if flow == "q":
                                # spread heads into zero-padded slabs for the
                                # score matmul lhsT, chunk by chunk
                                for h in range(NH):
                                    nc.vector.tensor_copy(
                                        out=qp_all[
                                            32 * h : 32 * h + HD,
                                            h * S + c0 : h * S + c0 + cw_,
                                        ],
                                        in_=qp_pack[32 * h : 32 * h + HD, sl],
                                    )

                    # ---- v^T blocks scaled by 0.25 (mean over heads) ----
                    for jt in range(NJT):
                        tp = pre_ps.tile([128, 65], F32, tag="tps")
                        nc.tensor.transpose(
                            tp[:, 0:C],
                            v_in[:, jt * 128 : (jt + 1) * 128],
                            ident[0:C, 0:C],
                        )
                        nc.vector.tensor_scalar(
                            out=vT_sb[:, jt * C : (jt + 1) * C],
                            in0=tp[:, 0:C],
                            scalar1=0.25,
                            scalar2=None,
                            op0=OP.mult,
                        )

            # ---------------- main attention loop ----------------
            if stage == "pre":
                nc.gpsimd.dma_start(
                    out=out_d,
                    in_=kp_pack[0:C, 0:S],
                )
                nc.compile()
                return nc

            with (
                tc.tile_pool(name="mainp", bufs=1) as mainp,
                tc.tile_pool(name="epool", bufs=2) as epool,
                tc.tile_pool(name="ppool", bufs=2) as ppool,
                tc.tile_pool(name="tpool", bufs=3) as tpool,
                tc.tile_pool(name="zpool", bufs=2) as zpool,
                tc.tile_pool(name="psB", bufs=2, space="PSUM") as psB,
                tc.tile_pool(name="psC", bufs=2, space="PSUM") as psC,
                tc.tile_pool(name="psO", bufs=2, space="PSUM") as psOp,
            ):
                # P^T storage: block jb holds [j 128, i S] at cols jb*S
                PT = mainp.tile([128, NJT * S], BF16)
                if stage == "noxp":
                    nc.vector.memset(PT, 0.0)

                def emit_pv(c):
                    i0, w = PVCH[c]
                    psO = psOp.tile([64, 512], F32, tag="O")
                    for jb in range(NJT):
                        nc.tensor.matmul(
                            psO[:, 0:w],
                            lhsT=vT_sb[:, jb * C : (jb + 1) * C],
                            rhs=PT[:, jb * S + i0 : jb * S + i0 + w],
                            start=(jb == 0),
                            stop=(jb == NJT - 1),
                        )
                    nc.vector.tensor_copy(
                        out=accT[:, i0 : i0 + w], in_=psO[0:C, 0:w]
                    )
                    nc.sync.dma_start(
                        out=out_d[:, i0 : i0 + w], in_=accT[:, i0 : i0 + w]
                    )

                use_acc = stage != "noacc"
                use_sch = stage != "nosch"
                for ib in range(NIB):
                    E = epool.tile([128, NH * S], BF16, tag="E")
                    P = ppool.tile([128, S], BF16, tag="P")
                    zsum = zpool.tile([128, NH * 3], F32, tag="zs")
                    rz = zpool.tile([128, NH], F32, tag="rz")
                    if not use_acc:
                        nc.gpsimd.memset(zsum, 1.0)
                    for h in range(NH):
                        lhs = qp_all[:, h * S + ib * 128 : h * S + (ib + 1) * 128]
                        for ci in range(2):
                            j0 = ci * 1024
                            ps = psB.tile([128, 1024], F32, tag="S")
                            for js in range(0, 1024, 512):
                                nc.tensor.matmul(
                                    ps[:, js : js + 512],
                                    lhsT=lhs,
                                    rhs=kp_pack[:, j0 + js : j0 + js + 512],
                                    start=True,
                                    stop=True,
                                )
                            nc.scalar.activation(
                                out=E[:, h * S + j0 : h * S + j0 + 1024],
                                in_=ps[:, 0:1024],
                                func=AF.Exp,
                                bias=zero_dep,
                                accum_out=(
                                    zsum[:, h * 3 + ci : h * 3 + ci + 1]
                                    if use_acc
                                    else None
                                ),
                            )
                        # tail columns on the vector engine (Schraudolph)
                        psc = psC.tile([128, 512], F32, tag="C")
                        nc.tensor.matmul(
                            psc[:, 0:JTAIL],
                            lhsT=lhs,
                            rhs=kp_pack[:, JACT:S],
                            start=True,
                            stop=True,
                        )
                        if use_sch:
                            t32 = tpool.tile([128, JTAIL], I32, tag="t32")
                            nc.vector.tensor_scalar(
                                out=t32,
                                in0=psc[:, 0:JTAIL],
                                scalar1=A32,
                                scalar2=B32,
                                op0=OP.mult,
                                op1=OP.add,
                            )
                            acc_t = (
                                zsum[:, h * 3 + 2 : h * 3 + 3]
                                if use_acc
                                else zpool.tile([128, 1], F32, tag="dump")
                            )
                            nc.vector.tensor_tensor_reduce(
                                out=E[:, h * S + JACT : h * S + S],
                                in0=t32.bitcast(F32),
                                in1=t32.bitcast(F32),
                                scale=1.0,
                                scalar=0.0,
                                op0=OP.max,
                                op1=OP.add,
                                accum_out=acc_t,
                            )
                        else:
                            nc.scalar.activation(
                                out=E[:, h * S + JACT : h * S + S],
                                in_=psc[:, 0:JTAIL],
                                func=AF.Exp,
                                bias=zero_dep,
                                accum_out=(
                                    zsum[:, h * 3 + 2 : h * 3 + 3]
                                    if use_acc
                                    else None
                                ),
                            )
                    # Z = sum of the three partials; rz = 1/Z
                    zr = zpool.tile([128, NH], F32, tag="zr")
                    nc.vector.tensor_reduce(
                        out=zr,
                        in_=zsum.rearrange("p (h c) -> p h c", c=3),
                        axis=AX.X,
                        op=OP.add,
                    )
                    nc.vector.reciprocal(rz, zr)
                    # P = sum_h E_h * rz_h
                    nc.vector.tensor_scalar(
                        out=P,
                        in0=E[:, 0:S],
                        scalar1=rz[:, 0:1],
                        scalar2=None,
                        op0=OP.mult,
                    )
                    for h in range(1, NH):
                        nc.vector.scalar_tensor_tensor(
                            out=P,
                            in0=E[:, h * S : (h + 1) * S],
                            scalar=rz[:, h : h + 1],
                            in1=P,
                            op0=OP.mult,
                            op1=OP.add,
                        )
                    # transpose P into PT via the DMA XBAR
                    if stage != "noxp":
                        for jb in range(NJT):
                            nc.sync.dma_start_transpose(
                                out=PT[
                                    :, jb * S + ib * 128 : jb * S + (ib + 1) * 128
                                ],
                                in_=P[:, jb * 128 : (jb + 1) * 128],
                            )
                    # PV chunks once their i-range is fully transposed
                    if ib in (5, 9, 13, 17):
                        emit_pv((ib - 5) // 4)
                emit_pv(4)

    nc.compile()
    return nc


_NC_CACHE = None


def _get_nc():
    global _NC_CACHE
    if _NC_CACHE is None:
        _NC_CACHE = build()
    return _NC_CACHE


def make_in_maps(q, k, v, conv_w, nq_w, nk_w, wq, bq, wk, bk):
    B = q.shape[0]
    f = lambda a, s: np.ascontiguousarray(a, dtype=np.float32).reshape(s)
    shared = {
        "conv_w": f(conv_w, (C, C * 9)),
        "wq": f(wq, (C, C)),
        "wk": f(wk, (C, C)),
        "bq": f(bq, (C, 1)),
        "bk": f(bk, (C, 1)),
        "nq_w": f(nq_w, (C, 1)),
        "nk_w": f(nk_w, (C, 1)),
    }
    return [
        {
            "q": f(q[b], (C, S)),
            "k": f(k[b], (C, S)),
            "v": f(v[b], (C, S)),
            **shared,
        }
        for b in range(B)
    ]


def run(in_maps, **kwargs):
    nc = _get_nc()
    return run_bass_kernel_spmd(nc, in_maps, core_ids=list(range(N_CORES)), **kwargs)


def kernel(q, k, v, conv_w, nq_w, nk_w, wq, bq, wk, bk):
    res = run(make_in_maps(q, k, v, conv_w, nq_w, nk_w, wq, bq, wk, bk))
    return np.stack(
        [res.results[b]["out"].reshape(C, S).T for b in range(q.shape[0])]
    )
